# revision 27
# baseline (speedup 1.0000x reference)
"""Trainium2 Bass kernel for nn_AttLayer (sliding-block attention encoder layer).

Sharding: 8 cores = 4 batches x 2 sequence halves (4096 frames each).
Each core gets its x1 slice with a 256-frame halo on both sides, computes
q/k/v projections, 8 blocks of windowed attention (block 512, window 1024),
relu + output projection locally. No collectives.

Mirror trick: sliding-window attention is reflection-symmetric (window =
block +-256), so the second-half cores get their sequence REVERSED on the
host. Every core then sees the zero-padded sequence edge as its LEFT halo
and real neighbor data as its RIGHT halo, letting one SPMD program skip all
work on the zero halo (k/v projection of cols [0,256) and the two padded
k-tiles of block 0's window) whose contribution to the reference output is
exactly zero (padded mask -> att*mb == 0, padded v == bias).

Device layout choices:
  - all matmul operands in BF16 (x, weights host-cast; q/k/pt/v/rl via
    PSUM-evacuation output dtype). PSUM accumulation stays fp32.
  - q, k stored [c=256(2 ptiles), Lext=4608] in SBUF.
  - v stored TRANSPOSED [Lext(36 ptiles), c3=256]  (computed directly as
    x^T @ Wv^T so no on-chip transpose is ever needed).
  - energy computed transposed: eT[k, q] = sum_c k[c,k] q[c,q]  -> the softmax
    log-mask bias lands on the partition dim, a perfect fit for the ACT
    engine's per-partition bias operand:  P = Exp(eT/16 + bias).
  - no max-subtraction in softmax (energies are O(10), exp is safe in fp32).
  - row sums over the exp tiles via a pairwise bf16 tree on DVE, then a
    [128,128] ones matmul that reduces partitions AND replicates the sum to
    all 128 partitions; reciprocal runs directly on that PSUM.
  - relu & normalization fused in one DVE op via the identity
    relu(o/s) = relu(o)*(1/s):   rl = (o max 0) * rb   (scalar_tensor_tensor).
  - an early dummy Exp on the ACT engine triggers the one-time
    ACT_TABLE_LOAD (~1.3us) during the DMA head instead of right when the
    first PSUM evacuation is needed.
  - PE warmup dummies (issued as early as possible: wrm is memset on the
    otherwise idle GpSimd engine which boots first) ramp the HAM clock gate
    during the DMA head; keep-warm dummies in the drain window hold the
    clock for the last block's output projection.
  - the last block's softmax tail / normrelu / output projection run in
    256-col halves so the final DVE chain overlaps PE work (shorter tail).
"""

import numpy as np

# problem constants (self-contained; must match the harness reference)
B, CIN, L = 4, 512, 8192
C, VD = 256, 512
BL, HALF = 512, 256
NCORES = 8
LCH = L // 2            # 4096 frames per core
LEXT = LCH + 2 * HALF   # 4608 with halo
NBLK = LCH // BL        # 8 local blocks
WS = BL + 2 * HALF      # 1024 window
NKT = WS // 128         # 8 k-tiles per window
NCH = LEXT // BL        # 9 x chunks
NVT = LEXT // 128       # 36 v^T partition tiles

_NC_CACHE = {}


def _build_nc(bv_zero, ab_paired=True):
    import concourse.bacc as bacc
    import concourse.mybir as mybir
    import concourse.tile as tile
    from contextlib import ExitStack

    f32 = mybir.dt.float32
    bf16 = mybir.dt.bfloat16
    rdt = bf16
    AF = mybir.ActivationFunctionType
    ALU = mybir.AluOpType

    nc = bacc.Bacc("TRN2", target_bir_lowering=False, debug=False,
                   num_devices=NCORES)

    # x and weights are host-packed so every DMA is a single contiguous
    # per-partition read (4KB packets instead of 4x1KB strided segments -
    # much faster on the cold DMA path at kernel start)
    x_d = nc.dram_tensor("x", [NCH, 128, 4, BL], rdt,
                         kind="ExternalInput").ap()
    wq_d = nc.dram_tensor("wq_t", [128, 4, C], rdt, kind="ExternalInput").ap()
    wk_d = nc.dram_tensor("wk_t", [128, 4, C], rdt, kind="ExternalInput").ap()
    wv_d = nc.dram_tensor("wv_t", [128, 4, C], rdt, kind="ExternalInput").ap()
    wo_d = nc.dram_tensor("wo_t", [128, 2, VD], rdt,
                          kind="ExternalInput").ap()
    bq_d = nc.dram_tensor("bq", [C, 1], f32, kind="ExternalInput").ap()
    bk_d = nc.dram_tensor("bk", [C, 1], f32, kind="ExternalInput").ap()
    bv_d = nc.dram_tensor("bv", [C, 1], f32, kind="ExternalInput").ap()
    ab_d = nc.dram_tensor("abias", [128, NBLK * NKT], f32,
                          kind="ExternalInput").ap()
    out_d = nc.dram_tensor("out", [VD, LCH], rdt, kind="ExternalOutput").ap()

    wq_r, wk_r, wv_r, wo_r = wq_d, wk_d, wv_d, wo_d
    bq_r = bq_d.rearrange("(m p) o -> p m o", p=128)    # [128, 2, 1]
    bk_r = bk_d.rearrange("(m p) o -> p m o", p=128)
    bv_r = bv_d.rearrange("(m p) o -> p m o", p=128)
    out_r = out_d.rearrange("(v p) l -> p v l", p=128)  # [128, 4, 4096]

    with tile.TileContext(nc) as tc:
        with ExitStack() as ctx:
            ctx.enter_context(nc.allow_low_precision(
                reason="bf16 matmul pipeline; fp32 PSUM accumulation"))
            sbc = ctx.enter_context(tc.tile_pool(name="sbc", bufs=1))  # constants
            sbp = ctx.enter_context(tc.tile_pool(name="sbp", bufs=1))  # persistent
            sbs = ctx.enter_context(tc.tile_pool(name="sbs", bufs=1))  # streaming
            ps = ctx.enter_context(tc.tile_pool(name="ps", bufs=1, space="PSUM"))

            dma = nc.sync.dma_start

            # warmup operand, memset on GpSimd (idle engine, earliest boot)
            wrm = sbc.tile([128, BL], bf16, tag="wrm", name="wrm")
            nc.gpsimd.memset(wrm[:], 0.0)
            # trigger the one-time ACT table load (~1.3us) during the boot
            # window. The bias must be a memset SBUF tile, NOT a float
            # constant: a float would allocate a const-AP whose init DMA
            # queues ahead of the x/weight transfers and delays them.
            zb = sbc.tile([128, 1], f32, tag="zb", name="zb")
            nc.gpsimd.memset(zb[:], 0.0)
            dact = sbc.tile([128, 1], f32, tag="dact", name="dact")
            nc.scalar.activation(dact[:], zb[:], AF.Exp, bias=zb[:],
                                 scale=1.0)

            wq = sbc.tile([128, 4, C], rdt, tag="wq", name="wq")
            wk = sbc.tile([128, 4, C], rdt, tag="wk", name="wk")
            wv = sbc.tile([128, 4, C], rdt, tag="wv", name="wv")
            xt0 = sbs.tile([128, 4, BL], rdt, tag="x", bufs=3, name="xt0")
            xt1 = sbs.tile([128, 4, BL], rdt, tag="x", bufs=3, name="xt1")
            bq = sbc.tile([128, 2, 1], f32, tag="bq", name="bq")
            bk = sbc.tile([128, 2, 1], f32, tag="bk", name="bk")
            dma(out=xt0[:], in_=x_d[0])
            dma(out=wq[:], in_=wq_r)
            dma(out=bq[:], in_=bq_r)
            dma(out=wk[:], in_=wk_r)
            dma(out=wv[:], in_=wv_r)
            dma(out=bk[:], in_=bk_r)
            dma(out=xt1[:], in_=x_d[1])
            xt2 = sbs.tile([128, 4, BL], rdt, tag="x", bufs=3, name="xt2")
            dma(out=xt2[:], in_=x_d[2])

            # PE warmup: dependency-free bf16 matmuls during the DMA head
            # so the HAM clock-gate reaches 8/8 before real work arrives.
            wps = ps.tile([128, BL], f32, tag="pp", bufs=2, name="wps")
            for i in range(10):
                nc.tensor.matmul(wps[:], wrm[:, 0:128], wrm[:], start=True,
                                 stop=True)

            def emit_late_consts():
                wo = sbc.tile([128, 2, VD], rdt, tag="wo", name="wo")
                dma(out=wo[:], in_=wo_r)
                bv = sbc.tile([128, 2, 1], f32, tag="bv", name="bv")
                dma(out=bv[:], in_=bv_r)
                ab = sbc.tile([128, NBLK * NKT], f32, tag="ab", name="ab")
                dma(out=ab[:], in_=ab_d)
                ones_f = sbc.tile([128, 128], f32, tag="ones_f", name="ones_f")
                nc.vector.memset(ones_f[:], 1.0)
                ones_m = sbc.tile([128, 128], rdt, tag="ones_m", name="ones_m")
                nc.vector.tensor_copy(ones_m[:], ones_f[:])
                return wo, bv, ab, ones_m

            q_sb = [sbp.tile([128, LEXT], rdt, tag=f"qsb{i}", name=f"qsb{i}")
                    for i in range(2)]
            k_sb = [sbp.tile([128, LEXT], rdt, tag=f"ksb{i}", name=f"ksb{i}")
                    for i in range(2)]
            # v^T tiles, paired: [128, 2, C] so the PSUM evacuation is one op
            vts = [sbp.tile([128, 2, C], rdt, tag=f"vt{i}", name=f"vt{i}")
                   for i in range(NVT // 2)]

            def mm(out_ap, lhsT, rhs, start, stop):
                nc.tensor.matmul(out_ap, lhsT, rhs, start=start, stop=stop)

            # paired-exp builds use 2-bank "ep" slots (2 bufs); the fallback
            # uses 1-bank "e" slots (4 bufs). One tag per build keeps the
            # PSUM budget at 8 banks: pp(2) + energy(4) + o0 + o1.
            ptag = "ep" if ab_paired else "e"
            pbufs = 2 if ab_paired else 4

            # ---------------- projections (streamed over 9 x-chunks) --------
            # chunk 0 skips cols [0, HALF) for k and v: that is the
            # zero-padded sequence edge (mirror trick) and no surviving
            # attention window reads it.
            def emit_chunk(c):
                if c == 0:
                    xt = xt0
                elif c == 1:
                    xt = xt1
                elif c == 2:
                    xt = xt2
                else:
                    xt = sbs.tile([128, 4, BL], rdt, tag="x", bufs=3,
                                  name=f"xt{c}")
                    dma(out=xt[:], in_=x_d[c])
                # q is only needed on extended cols [HALF, LEXT-HALF)
                qlo = max(c * BL, HALF) - c * BL
                qhi = min((c + 1) * BL, LEXT - HALF) - c * BL
                klo = HALF if c == 0 else 0
                for o in range(2):
                    pq = ps.tile([128, BL], f32, tag="pp", bufs=2,
                                 name=f"pq{c}_{o}")
                    for r in range(4):
                        mm(pq[:, 0:qhi - qlo],
                           wq[:, r, o * 128:(o + 1) * 128],
                           xt[:, r, qlo:qhi], r == 0, r == 3)
                    # split the two q evacuations across scalar/vector so
                    # the pp-ring drains in parallel
                    if o == 0:
                        nc.scalar.activation(
                            q_sb[o][:, c * BL + qlo:c * BL + qhi],
                            pq[:, 0:qhi - qlo], AF.Identity,
                            bias=bq[:, o, :], scale=1.0)
                    else:
                        nc.vector.tensor_scalar_add(
                            q_sb[o][:, c * BL + qlo:c * BL + qhi],
                            pq[:, 0:qhi - qlo], bq[:, o, :])
                # k PSUM groups live on the ep ring (free during the
                # projection phase): this decouples them from the q groups'
                # pp ring so neither waits on the other's evacuation.
                for o in range(2):
                    pk = ps.tile([128, BL], f32, tag=ptag, bufs=pbufs,
                                 name=f"pk{c}_{o}")
                    for r in range(4):
                        mm(pk[:, 0:BL - klo],
                           wk[:, r, o * 128:(o + 1) * 128],
                           xt[:, r, klo:BL], r == 0, r == 3)
                    nc.vector.tensor_scalar_add(
                        k_sb[o][:, c * BL + klo:(c + 1) * BL],
                        pk[:, 0:BL - klo], bk[:, o, :])
                for lp in range(2):
                    if c == 0 and lp == 0:
                        continue  # zero-halo v tiles: never read
                    pv = ps.tile([128, 2, C], f32, tag=ptag, bufs=pbufs,
                                 name=f"pv{c}_{lp}")
                    for j in range(2):
                        lt = lp * 2 + j
                        for r in range(4):
                            mm(pv[:, j, :], xt[:, r, lt * 128:(lt + 1) * 128],
                               wv[:, r, :], r == 0, r == 3)
                    nc.vector.tensor_copy(vts[c * 2 + lp][:], pv[:])

            emit_chunk(0)
            wo, bv, ab, ones_m = emit_late_consts()
            emit_chunk(1)
            emit_chunk(2)

            # ---------------- attention (software-pipelined blocks) ---------
            OPS, SPS, RBS, ORL, PTS = {}, {}, {}, {}, {}
            HV = BL // 2  # half width for the last block's tail split

            def qk_pair(b, kp, split=False):
                # 2-bank energy tile; one Exp ACT covers both k-tiles
                # (the log-mask bias is per-partition and pairwise-equal
                # for every mask setup_inputs produces - host-verified)
                pts = PTS[b]
                pe = ps.tile([128, 2, BL], f32, tag="ep", bufs=2,
                             name=f"ep{b}_{kp}")
                for j in range(2):
                    kt = 2 * kp + j
                    for ct in range(2):
                        mm(pe[:, j, :],
                           k_sb[ct][:, b * BL + kt * 128:
                                    b * BL + (kt + 1) * 128],
                           q_sb[ct][:, HALF + b * BL:HALF + (b + 1) * BL],
                           ct == 0, ct == 1)
                bias = ab[:, b * NKT + 2 * kp:b * NKT + 2 * kp + 1]
                if split:
                    # two half-width exps shorten the last block's
                    # critical path into the softmax tail
                    for h in range(2):
                        nc.scalar.activation(
                            pts[:, 2 * kp:2 * kp + 2, h * HV:(h + 1) * HV],
                            pe[:, :, h * HV:(h + 1) * HV], AF.Exp,
                            bias=bias, scale=1.0 / 16.0)
                else:
                    nc.scalar.activation(
                        pts[:, 2 * kp:2 * kp + 2, :], pe[:], AF.Exp,
                        bias=bias, scale=1.0 / 16.0)

            def qk_single(b, kt):
                pts = PTS[b]
                pe = ps.tile([128, BL], f32, tag="e", bufs=4,
                             name=f"e{b}_{kt}")
                for ct in range(2):
                    mm(pe[:],
                       k_sb[ct][:, b * BL + kt * 128:
                                b * BL + (kt + 1) * 128],
                       q_sb[ct][:, HALF + b * BL:HALF + (b + 1) * BL],
                       ct == 0, ct == 1)
                nc.scalar.activation(
                    pts[:, kt, :], pe[:], AF.Exp,
                    bias=ab[:, b * NKT + kt:b * NKT + kt + 1],
                    scale=1.0 / 16.0)

            def emit_attn_head(b):
                # first two qk pairs of block b, emitted during block b-1's
                # av phase: their exps run on the ACT engine while the PE
                # finishes b-1, so b's first av never waits on an exp.
                PTS[b] = sbs.tile([128, NKT, BL], rdt, tag="pt", bufs=2,
                                  name=f"pt{b}")
                if ab_paired:
                    for kp in ((1, 2) if b == 0 else (0, 1)):
                        qk_pair(b, kp)
                else:
                    kt0 = 2 if b == 0 else 0
                    for kt in range(kt0, kt0 + 4):
                        qk_single(b, kt)

            def emit_attn_body(b):
                first = b == 0
                last = b == NBLK - 1 and ab_paired
                kt0 = 2 if first else 0  # padded k-tiles of block 0 skipped
                pts = PTS[b]
                o0 = ps.tile([128, BL], f32, tag="o0", bufs=1, name=f"o0_{b}")
                o1 = ps.tile([128, BL], f32, tag="o1", bufs=1, name=f"o1_{b}")

                def av(kt):
                    vtt = vts[(b * 4 + kt) // 2]
                    j = (b * 4 + kt) % 2
                    pt = pts[:, kt, :]
                    mm(o0[:], vtt[:, j, 0:128], pt, kt == kt0, kt == NKT - 1)
                    mm(o1[:], vtt[:, j, 128:256], pt, kt == kt0, kt == NKT - 1)

                if ab_paired:
                    if first:
                        av(2)
                        av(3)
                        qk_pair(b, 3)
                        for kt in range(4, NKT):
                            av(kt)
                    else:
                        av(0)
                        qk_pair(b, 2)
                        av(1)
                        av(2)
                        qk_pair(b, 3, split=last)
                        for kt in range(3, NKT):
                            av(kt)
                else:
                    for kt in range(kt0, NKT):
                        av(kt)
                        if kt + 4 < NKT:
                            qk_single(b, kt + 4)

                # pairwise bf16 tree for the row sums, split in halves:
                # the a-side (kt 0-3) only needs the first two exp pairs, so
                # it runs early (during av) and the post-exp critical path is
                # just the b-side + the final combine.
                a2 = sbs.tile([128, BL], rdt, tag="t2", bufs=2,
                              name=f"a2_{b}")
                if last:
                    # last block: fold three pairs (kt 0-5) into the a-side
                    # so only the kt 6/7 pair remains on the post-exp path,
                    # and run that path in halves; the sum lands in plane 0
                    # of an ep-ring-shaped PSUM tile (the pp ring stays
                    # exclusive to outproj(b-1) at the end).
                    a1 = sbs.tile([128, 3, BL], rdt, tag="t4", bufs=2,
                                  name=f"a1_{b}")
                    nc.vector.tensor_add(a1[:], pts[:, 0:6:2, :],
                                         pts[:, 1:6:2, :])
                    a2a = sbs.tile([128, BL], rdt, tag="t2a", bufs=2,
                                   name=f"a2a_{b}")
                    nc.vector.tensor_add(a2a[:], a1[:, 0, :], a1[:, 1, :])
                    nc.vector.tensor_add(a2[:], a2a[:], a1[:, 2, :])
                    sp = ps.tile([128, 2, BL], f32, tag="ep", bufs=2,
                                 name=f"s{b}")
                    for h in range(2):
                        hs = slice(h * HV, (h + 1) * HV)
                        b2 = sbs.tile([128, HV], rdt, tag="t2b", bufs=2,
                                      name=f"b2_{b}_{h}")
                        nc.vector.tensor_add(b2[:], pts[:, 6, hs],
                                             pts[:, 7, hs])
                        sar = sbs.tile([128, HV], rdt, tag="sar", bufs=2,
                                       name=f"sar{b}_{h}")
                        nc.vector.tensor_add(sar[:], a2[:, hs], b2[:])
                        mm(sp[:, 0, hs], ones_m[:], sar[:], True, True)
                elif first:
                    nc.vector.tensor_add(a2[:], pts[:, 2, :], pts[:, 3, :])
                else:
                    a1 = sbs.tile([128, 2, BL], rdt, tag="t4", bufs=2,
                                  name=f"a1_{b}")
                    nc.vector.tensor_add(a1[:], pts[:, 0:4:2, :],
                                         pts[:, 1:4:2, :])
                    nc.vector.tensor_add(a2[:], a1[:, 0, :], a1[:, 1, :])
                if not last:
                    b1 = sbs.tile([128, 2, BL], rdt, tag="t4b", bufs=2,
                                  name=f"b1_{b}")
                    nc.vector.tensor_add(b1[:], pts[:, 4:NKT:2, :],
                                         pts[:, 5:NKT:2, :])
                    b2 = sbs.tile([128, BL], rdt, tag="t2b", bufs=2,
                                  name=f"b2_{b}")
                    nc.vector.tensor_add(b2[:], b1[:, 0, :], b1[:, 1, :])
                    sar = sbs.tile([128, BL], rdt, tag="sar", bufs=2,
                                   name=f"sar{b}")
                    nc.vector.tensor_add(sar[:], a2[:], b2[:])
                    SAR[b] = sar
                else:
                    SPS[b] = sp
                OPS[b] = (o0, o1)

            def emit_sum(b):
                # partition all-reduce on the otherwise idle GpSimd engine:
                # reduces across partitions AND replicates the row-sum to
                # all 128 partitions (fp32 accumulation), freeing the PE of
                # the per-block [128,128] ones matmul. The last block keeps
                # the matmul path (shorter latency for the tail).
                from concourse.bass_isa import ReduceOp
                spf = sbs.tile([128, BL], f32, tag="spf", bufs=2,
                               name=f"s{b}")
                nc.gpsimd.partition_all_reduce(spf[:], SAR[b][:], 128,
                                               ReduceOp.add)
                SPS[b] = spf

            SAR = {}

            def emit_finA(b, h=None):
                hs = slice(0, BL) if h is None else slice(h * HV, (h + 1) * HV)
                if h in (None, 0):
                    rb = sbs.tile([128, BL], f32, tag="rbs", bufs=2,
                                  name=f"rb{b}")
                    RBS[b] = rb
                tail = b == NBLK - 1 and ab_paired
                src = SPS[b][:, 0, hs] if tail else SPS[b][:, hs]
                nc.vector.reciprocal_approx_fast(RBS[b][:, hs], src)

            def emit_normrelu(b, h=None):
                hs = slice(0, BL) if h is None else slice(h * HV, (h + 1) * HV)
                if h in (None, 0):
                    ORL[b] = [
                        sbs.tile([128, BL], rdt, tag=f"rl{m}", bufs=2,
                                 name=f"rl{b}_{m}")
                        for m in range(2)]
                for m in range(2):
                    rl = ORL[b][m]
                    if bv_zero:
                        # relu(o/s + 0) == relu(o) * (1/s)   (s > 0)
                        nc.vector.scalar_tensor_tensor(
                            rl[:, hs], OPS[b][m][:, hs], 0.0, RBS[b][:, hs],
                            ALU.max, ALU.mult)
                    else:
                        on = sbs.tile([128, BL], f32, tag=f"on{m}", bufs=2,
                                      name=f"on{b}_{m}_{h}")
                        nc.vector.tensor_mul(on[:, hs], OPS[b][m][:, hs],
                                             RBS[b][:, hs])
                        nc.vector.tensor_scalar(
                            rl[:, hs], on[:, hs], bv[:, m, :], 0.0,
                            ALU.add, ALU.max)

            def emit_outproj(b):
                ob = OB[b] = sbs.tile([128, 4, BL], rdt, tag="ob", bufs=2,
                                      name=f"ob{b}")
                for v in range(4):
                    po = ps.tile([128, BL], f32, tag="pp", bufs=2,
                                 name=f"po{b}_{v}")
                    for m in range(2):
                        mm(po[:], wo[:, m, v * 128:(v + 1) * 128],
                           ORL[b][m][:], m == 0, m == 1)
                    # bo is added on the host; spread the evacuation copies
                    # across engines (gpsimd cannot read PSUM), and DMA each
                    # v-slice as it lands so the tail transfer is small
                    # for the second-to-last block ALL copies go to the
                    # scalar engine: the DVE must stay clear for the last
                    # block's softmax tail chain.
                    if v in (1, 3) or (b == NBLK - 2 and ab_paired):
                        nc.scalar.copy(ob[:, v, :], po[:])
                    else:
                        nc.vector.tensor_copy(ob[:, v, :], po[:])
                    dma(out=out_r[:, v:v + 1, b * BL:(b + 1) * BL],
                        in_=ob[:, v:v + 1, :])

            def emit_tail_outproj_half(b, h):
                # last block: each (v-pair, half) quarter gets its OWN
                # ep-ring-shaped PSUM tile (shared tiles would create
                # tile-level false deps between the h0 copies and the h1
                # matmuls). h0 copies go to the scalar engine (free after
                # the exps), h1 copies to the DVE (free after its softmax
                # chain).
                hs = slice(h * HV, (h + 1) * HV)
                ob = OB[b]
                for vp in range(2):
                    po = ps.tile([128, 2, BL], f32, tag="ep", bufs=2,
                                 name=f"tpo{b}_{h}_{vp}")
                    for j in range(2):
                        v = vp * 2 + j
                        for m in range(2):
                            mm(po[:, j, 0:HV],
                               wo[:, m, v * 128:(v + 1) * 128],
                               ORL[b][m][:, hs], m == 0, m == 1)
                    if h == 0:
                        nc.scalar.copy(ob[:, 2 * vp:2 * vp + 2, hs],
                                       po[:, :, 0:HV])
                    else:
                        nc.vector.tensor_copy(ob[:, 2 * vp:2 * vp + 2, hs],
                                              po[:, :, 0:HV])
                    dma(out=out_r[:, 2 * vp:2 * vp + 2, b * BL + hs.start:
                                  b * BL + hs.stop],
                        in_=ob[:, 2 * vp:2 * vp + 2, hs])

            OB = {}

            # block 0's qk head is emitted mid-projections (its k/q inputs
            # are ready after chunk 2) so its exps are long done when the
            # attention phase starts.
            emit_attn_head(0)
            for c in range(3, NCH):
                emit_chunk(c)

            for step in range(NBLK + 1):
                if step == NBLK:
                    # pipeline drain: keep the PE pstate hot with dummies so
                    # the last outproj runs at full clock instead of the
                    # cold 2-3x-slower rate. The first dummy reads the
                    # previous block's evacuated output so the scheduler
                    # cannot hoist the drain ahead of that outproj.
                    dps = ps.tile([128, BL], f32, tag="pp", bufs=2,
                                  name="dummy")
                    ndrain = (8, 4) if ab_paired else (16, 10)
                    anchor = OB[NBLK - 2][:, 0, 0:256]
                    for i in range(ndrain[0]):
                        nc.tensor.matmul(dps[:, 0:256], wrm[:, 0:128],
                                         anchor if i == 0 else wrm[:, 0:256],
                                         start=True, stop=True)
                    for i in range(ndrain[1]):
                        nc.tensor.matmul(dps[:, 0:64], wrm[:, 0:128],
                                         wrm[:, 0:64], start=True, stop=True)
                if 1 <= step <= NBLK:
                    if step == NBLK and ab_paired:
                        # last block: halves-pipelined tail. Both recips are
                        # queued first (so the sum tile's ep-ring slot frees
                        # early), then the stts; the h0 outproj matmuls and
                        # scalar copies overlap the DVE's h1 chain.
                        b = step - 1
                        OB[b] = sbs.tile([128, 4, BL], rdt, tag="ob",
                                         bufs=2, name=f"ob{b}")
                        emit_finA(b, 0)
                        emit_finA(b, 1)
                        emit_normrelu(b, 0)
                        emit_normrelu(b, 1)
                        emit_tail_outproj_half(b, 0)
                        emit_tail_outproj_half(b, 1)
                    else:
                        emit_normrelu(step - 1)
                if step < NBLK:
                    emit_attn_body(step)
                    if step + 1 < NBLK:
                        emit_attn_head(step + 1)
                    if not (step == NBLK - 1 and ab_paired):
                        emit_sum(step)
                if step < NBLK:
                    if step == NBLK - 1 and ab_paired:
                        pass  # finA folded into the tail halves above
                    else:
                        emit_finA(step)
                if 1 <= step <= NBLK:
                    if step == NBLK and ab_paired:
                        pass  # outproj folded into the tail halves above
                    else:
                        emit_outproj(step - 1)

    nc.compile()
    return nc


def get_nc(bv_zero=True, ab_paired=True):
    key = ("bf16", bv_zero, ab_paired)
    if key not in _NC_CACHE:
        _NC_CACHE[key] = _build_nc(bv_zero, ab_paired)
    return _NC_CACHE[key]


def make_core_inputs(inputs):
    """Split full inputs into 8 per-core input maps.

    Cores 2h+1 (second sequence half) get their slice REVERSED along L so
    the zero-padded sequence edge is the left halo on every core (the
    sliding-window attention is reflection-symmetric). assemble_output
    un-reverses.
    """
    import ml_dtypes
    bf16 = ml_dtypes.bfloat16

    x1 = np.asarray(inputs["x1"], dtype=np.float32)
    mask = np.asarray(inputs["mask"], dtype=np.float32)
    def pack_w(w, groups):
        # [Cin, Cout] -> transposed, partition-packed [128, groups, Cout]
        wt = np.asarray(w, np.float32).T.astype(bf16)
        return np.ascontiguousarray(
            wt.reshape(groups, 128, wt.shape[1]).transpose(1, 0, 2))

    wq_t = pack_w(inputs["Wq"], 4)
    wk_t = pack_w(inputs["Wk"], 4)
    wv_t = pack_w(inputs["Wv"], 4)
    wo_t = pack_w(inputs["Wo"], 2)
    bq = np.asarray(inputs["bq"], np.float32).reshape(C, 1)
    bk = np.asarray(inputs["bk"], np.float32).reshape(C, 1)
    bv = np.asarray(inputs["bv"], np.float32).reshape(C, 1)

    in_maps = []
    for core in range(NCORES):
        b, h = divmod(core, 2)
        # core-extended input cols j in [0, LEXT); j==HALF is the first
        # owned frame. h==0: frame(j) = j - HALF; h==1 (reversed):
        # frame(j) = (L - 1 + HALF) - j. Out-of-range -> zero pad (always
        # the LEFT halo j < HALF thanks to the mirror trick).
        xe = np.zeros((CIN, LEXT), bf16)
        me = np.zeros((LEXT,), np.float32)
        if h == 0:
            xe[:, HALF:] = x1[b, :, 0:LCH + HALF].astype(bf16)
            me[HALF:] = mask[b, 0, 0:LCH + HALF]
        else:
            xe[:, HALF:] = x1[b, :, LCH - HALF:L][:, ::-1].astype(bf16)
            me[HALF:] = mask[b, 0, LCH - HALF:L][::-1]
        lbc = np.log(me + np.float32(1e-6))
        # pack chunk-contiguous: [NCH, 128, 4, BL]
        xe = np.ascontiguousarray(
            xe.reshape(4, 128, NCH, BL).transpose(2, 1, 0, 3))
        ab = np.empty((128, NBLK * NKT), np.float32)
        for blk in range(NBLK):
            w = lbc[blk * BL:blk * BL + WS]
            ab[:, blk * NKT:(blk + 1) * NKT] = w.reshape(NKT, 128).T
        in_maps.append({
            "x": xe, "wq_t": wq_t, "wk_t": wk_t, "wv_t": wv_t, "wo_t": wo_t,
            "bq": bq, "bk": bk, "bv": bv, "abias": ab,
        })
    return in_maps


def assemble_output(results, bo):
    out = np.empty((B, VD, L), np.float32)
    bo_col = np.asarray(bo, np.float32).reshape(VD, 1)
    for core in range(NCORES):
        b, h = divmod(core, 2)
        o = results[core]["out"].astype(np.float32)
        if h == 1:
            o = o[:, ::-1]
        out[b, :, h * LCH:(h + 1) * LCH] = o + bo_col
    return out


LAST_RESULT = None


def kernel(**inputs):
    global LAST_RESULT
    from concourse.bass_utils import run_bass_kernel_spmd

    bv_zero = bool(np.all(np.asarray(inputs["bv"]) == 0.0))
    in_maps = make_core_inputs(inputs)
    ab_paired = all(
        np.array_equal(m["abias"][:, 0::2], m["abias"][:, 1::2])
        for m in in_maps)
    nc = get_nc(bv_zero, ab_paired)
    res = run_bass_kernel_spmd(nc, in_maps, list(range(NCORES)))
    LAST_RESULT = res
    return assemble_output(res.results, inputs["bo"])


# revision 29
# speedup vs baseline: 1.2195x; 1.2195x over previous
"""Trainium2 Bass kernel for nn_AttLayer (sliding-block attention encoder layer).

Sharding: 8 cores = 4 batches x 2 sequence halves (4096 frames each).
Each core gets its x1 slice with a 256-frame halo on both sides, computes
q/k/v projections, 8 blocks of windowed attention (block 512, window 1024),
relu + output projection locally. No collectives.

Mirror trick: sliding-window attention is reflection-symmetric (window =
block +-256), so the second-half cores get their sequence REVERSED on the
host. Every core then sees the zero-padded sequence edge as its LEFT halo
and real neighbor data as its RIGHT halo, letting one SPMD program skip all
work on the zero halo (k/v projection of cols [0,256) and the two padded
k-tiles of block 0's window) whose contribution to the reference output is
exactly zero (padded mask -> att*mb == 0, padded v == bias).

Device layout choices:
  - all matmul operands in BF16 (x, weights host-cast; q/k/pt/v/rl via
    PSUM-evacuation output dtype). PSUM accumulation stays fp32.
  - q, k stored [c=256(2 ptiles), Lext=4608] in SBUF.
  - v stored TRANSPOSED [Lext(36 ptiles), c3=256]  (computed directly as
    x^T @ Wv^T so no on-chip transpose is ever needed).
  - energy computed transposed: eT[k, q] = sum_c k[c,k] q[c,q]  -> the softmax
    log-mask bias lands on the partition dim, a perfect fit for the ACT
    engine's per-partition bias operand:  P = Exp(eT/16 + bias).
  - no max-subtraction in softmax (energies are O(10), exp is safe in fp32).
  - row sums over the exp tiles via a pairwise bf16 tree on DVE, then a
    [128,128] ones matmul that reduces partitions AND replicates the sum to
    all 128 partitions; reciprocal runs directly on that PSUM.
  - relu & normalization fused in one DVE op via the identity
    relu(o/s) = relu(o)*(1/s):   rl = (o max 0) * rb   (scalar_tensor_tensor).
  - an early dummy Exp on the ACT engine triggers the one-time
    ACT_TABLE_LOAD (~1.3us) during the DMA head instead of right when the
    first PSUM evacuation is needed.
  - PE warmup dummies (issued as early as possible: wrm is memset on the
    otherwise idle GpSimd engine which boots first) ramp the HAM clock gate
    during the DMA head; keep-warm dummies in the drain window hold the
    clock for the last block's output projection.
  - the last block's softmax tail / normrelu / output projection run in
    256-col halves so the final DVE chain overlaps PE work (shorter tail).
"""

import numpy as np

# problem constants (self-contained; must match the harness reference)
B, CIN, L = 4, 512, 8192
C, VD = 256, 512
BL, HALF = 512, 256
NCORES = 8
LCH = L // 2            # 4096 frames per core
LEXT = LCH + 2 * HALF   # 4608 with halo
NBLK = LCH // BL        # 8 local blocks
WS = BL + 2 * HALF      # 1024 window
NKT = WS // 128         # 8 k-tiles per window
NCH = LEXT // BL        # 9 x chunks
NVT = LEXT // 128       # 36 v^T partition tiles

_NC_CACHE = {}


def _build_nc(bv_zero, ab_paired=True):
    import concourse.bacc as bacc
    import concourse.mybir as mybir
    import concourse.tile as tile
    from contextlib import ExitStack

    f32 = mybir.dt.float32
    bf16 = mybir.dt.bfloat16
    rdt = bf16
    AF = mybir.ActivationFunctionType
    ALU = mybir.AluOpType

    nc = bacc.Bacc("TRN2", target_bir_lowering=False, debug=False,
                   num_devices=NCORES)

    # x and weights are host-packed so every DMA is a single contiguous
    # per-partition read (4KB packets instead of 4x1KB strided segments -
    # much faster on the cold DMA path at kernel start)
    x_d = nc.dram_tensor("x", [NCH, 128, 4, BL], rdt,
                         kind="ExternalInput").ap()
    wq_d = nc.dram_tensor("wq_t", [128, 4, C], rdt, kind="ExternalInput").ap()
    wk_d = nc.dram_tensor("wk_t", [128, 4, C], rdt, kind="ExternalInput").ap()
    wv_d = nc.dram_tensor("wv_t", [128, 4, C], rdt, kind="ExternalInput").ap()
    wo_d = nc.dram_tensor("wo_t", [128, 2, VD], rdt,
                          kind="ExternalInput").ap()
    bq_d = nc.dram_tensor("bq", [C, 1], f32, kind="ExternalInput").ap()
    bk_d = nc.dram_tensor("bk", [C, 1], f32, kind="ExternalInput").ap()
    bv_d = nc.dram_tensor("bv", [C, 1], f32, kind="ExternalInput").ap()
    ab_d = nc.dram_tensor("abias", [128, NBLK * NKT], f32,
                          kind="ExternalInput").ap()
    out_d = nc.dram_tensor("out", [VD, LCH], rdt, kind="ExternalOutput").ap()

    wq_r, wk_r, wv_r, wo_r = wq_d, wk_d, wv_d, wo_d
    bq_r = bq_d.rearrange("(m p) o -> p m o", p=128)    # [128, 2, 1]
    bk_r = bk_d.rearrange("(m p) o -> p m o", p=128)
    bv_r = bv_d.rearrange("(m p) o -> p m o", p=128)
    out_r = out_d.rearrange("(v p) l -> p v l", p=128)  # [128, 4, 4096]

    with tile.TileContext(nc) as tc:
        with ExitStack() as ctx:
            ctx.enter_context(nc.allow_low_precision(
                reason="bf16 matmul pipeline; fp32 PSUM accumulation"))
            sbc = ctx.enter_context(tc.tile_pool(name="sbc", bufs=1))  # constants
            sbp = ctx.enter_context(tc.tile_pool(name="sbp", bufs=1))  # persistent
            sbs = ctx.enter_context(tc.tile_pool(name="sbs", bufs=1))  # streaming
            ps = ctx.enter_context(tc.tile_pool(name="ps", bufs=1, space="PSUM"))

            dma = nc.sync.dma_start

            # warmup operand, memset on GpSimd (idle engine, earliest boot)
            wrm = sbc.tile([128, BL], bf16, tag="wrm", name="wrm")
            nc.gpsimd.memset(wrm[:], 0.0)
            # trigger the one-time ACT table load (~1.3us) during the boot
            # window. The bias must be a memset SBUF tile, NOT a float
            # constant: a float would allocate a const-AP whose init DMA
            # queues ahead of the x/weight transfers and delays them.
            zb = sbc.tile([128, 1], f32, tag="zb", name="zb")
            nc.gpsimd.memset(zb[:], 0.0)
            dact = sbc.tile([128, 1], f32, tag="dact", name="dact")
            nc.scalar.activation(dact[:], zb[:], AF.Exp, bias=zb[:],
                                 scale=1.0)

            wq = sbc.tile([128, 4, C], rdt, tag="wq", name="wq")
            wk = sbc.tile([128, 4, C], rdt, tag="wk", name="wk")
            wv = sbc.tile([128, 4, C], rdt, tag="wv", name="wv")
            xt0 = sbs.tile([128, 4, BL], rdt, tag="x", bufs=3, name="xt0")
            xt1 = sbs.tile([128, 4, BL], rdt, tag="x", bufs=3, name="xt1")
            bq = sbc.tile([128, 2, 1], f32, tag="bq", name="bq")
            bk = sbc.tile([128, 2, 1], f32, tag="bk", name="bk")
            # the first transfers are issued from the GpSimd queue, whose
            # sequencer boots ~2us before the Sync engine's; xt0 goes in
            # halves so the first q matmuls can start on half 0.
            gdma = nc.gpsimd.dma_start
            gdma(out=wq[:], in_=wq_r)
            gdma(out=bq[:], in_=bq_r)
            gdma(out=xt0[:, 0:2, :], in_=x_d[0, :, 0:2, :])
            gdma(out=xt0[:, 2:4, :], in_=x_d[0, :, 2:4, :])
            dma(out=wk[:], in_=wk_r)
            dma(out=wv[:], in_=wv_r)
            dma(out=bk[:], in_=bk_r)
            dma(out=xt1[:], in_=x_d[1])
            xt2 = sbs.tile([128, 4, BL], rdt, tag="x", bufs=3, name="xt2")
            dma(out=xt2[:], in_=x_d[2])

            # PE warmup: dependency-free bf16 matmuls so the HAM clock-gate
            # ramps during the (now shorter) DMA head.
            wps = ps.tile([128, BL], f32, tag="pp", bufs=2, name="wps")
            for i in range(4):
                nc.tensor.matmul(wps[:], wrm[:, 0:128], wrm[:], start=True,
                                 stop=True)

            def emit_late_consts():
                wo = sbc.tile([128, 2, VD], rdt, tag="wo", name="wo")
                dma(out=wo[:], in_=wo_r)
                bv = sbc.tile([128, 2, 1], f32, tag="bv", name="bv")
                dma(out=bv[:], in_=bv_r)
                ab = sbc.tile([128, NBLK * NKT], f32, tag="ab", name="ab")
                dma(out=ab[:], in_=ab_d)
                ones_f = sbc.tile([128, 128], f32, tag="ones_f", name="ones_f")
                nc.vector.memset(ones_f[:], 1.0)
                ones_m = sbc.tile([128, 128], rdt, tag="ones_m", name="ones_m")
                nc.vector.tensor_copy(ones_m[:], ones_f[:])
                return wo, bv, ab, ones_m

            q_sb = [sbp.tile([128, LEXT], rdt, tag=f"qsb{i}", name=f"qsb{i}")
                    for i in range(2)]
            k_sb = [sbp.tile([128, LEXT], rdt, tag=f"ksb{i}", name=f"ksb{i}")
                    for i in range(2)]
            # v^T tiles, paired: [128, 2, C] so the PSUM evacuation is one op
            vts = [sbp.tile([128, 2, C], rdt, tag=f"vt{i}", name=f"vt{i}")
                   for i in range(NVT // 2)]

            def mm(out_ap, lhsT, rhs, start, stop):
                nc.tensor.matmul(out_ap, lhsT, rhs, start=start, stop=stop)

            # paired-exp builds use 2-bank "ep" slots (2 bufs); the fallback
            # uses 1-bank "e" slots (4 bufs). One tag per build keeps the
            # PSUM budget at 8 banks: pp(2) + energy(4) + o0 + o1.
            ptag = "ep" if ab_paired else "e"
            pbufs = 2 if ab_paired else 4

            # ---------------- projections (streamed over 9 x-chunks) --------
            # chunk 0 skips cols [0, HALF) for k and v: that is the
            # zero-padded sequence edge (mirror trick) and no surviving
            # attention window reads it.
            def emit_chunk(c):
                if c == 0:
                    xt = xt0
                elif c == 1:
                    xt = xt1
                elif c == 2:
                    xt = xt2
                else:
                    xt = sbs.tile([128, 4, BL], rdt, tag="x", bufs=3,
                                  name=f"xt{c}")
                    dma(out=xt[:], in_=x_d[c])
                # q is only needed on extended cols [HALF, LEXT-HALF)
                qlo = max(c * BL, HALF) - c * BL
                qhi = min((c + 1) * BL, LEXT - HALF) - c * BL
                klo = HALF if c == 0 else 0
                for o in range(2):
                    pq = ps.tile([128, BL], f32, tag="pp", bufs=2,
                                 name=f"pq{c}_{o}")
                    for r in range(4):
                        mm(pq[:, 0:qhi - qlo],
                           wq[:, r, o * 128:(o + 1) * 128],
                           xt[:, r, qlo:qhi], r == 0, r == 3)
                    # split the two q evacuations across scalar/vector so
                    # the pp-ring drains in parallel
                    if o == 0:
                        nc.scalar.activation(
                            q_sb[o][:, c * BL + qlo:c * BL + qhi],
                            pq[:, 0:qhi - qlo], AF.Identity,
                            bias=bq[:, o, :], scale=1.0)
                    else:
                        nc.vector.tensor_scalar_add(
                            q_sb[o][:, c * BL + qlo:c * BL + qhi],
                            pq[:, 0:qhi - qlo], bq[:, o, :])
                # k PSUM groups live on the ep ring (free during the
                # projection phase): this decouples them from the q groups'
                # pp ring so neither waits on the other's evacuation.
                for o in range(2):
                    pk = ps.tile([128, BL], f32, tag=ptag, bufs=pbufs,
                                 name=f"pk{c}_{o}")
                    for r in range(4):
                        mm(pk[:, 0:BL - klo],
                           wk[:, r, o * 128:(o + 1) * 128],
                           xt[:, r, klo:BL], r == 0, r == 3)
                    nc.vector.tensor_scalar_add(
                        k_sb[o][:, c * BL + klo:(c + 1) * BL],
                        pk[:, 0:BL - klo], bk[:, o, :])
                for lp in range(2):
                    if c == 0 and lp == 0:
                        continue  # zero-halo v tiles: never read
                    pv = ps.tile([128, 2, C], f32, tag=ptag, bufs=pbufs,
                                 name=f"pv{c}_{lp}")
                    for j in range(2):
                        lt = lp * 2 + j
                        for r in range(4):
                            mm(pv[:, j, :], xt[:, r, lt * 128:(lt + 1) * 128],
                               wv[:, r, :], r == 0, r == 3)
                    nc.vector.tensor_copy(vts[c * 2 + lp][:], pv[:])

            emit_chunk(0)
            wo, bv, ab, ones_m = emit_late_consts()
            emit_chunk(1)
            emit_chunk(2)

            # ---------------- attention (software-pipelined blocks) ---------
            OPS, SPS, RBS, ORL, PTS = {}, {}, {}, {}, {}
            HV = BL // 2  # half width for the last block's tail split

            def qk_pair(b, kp, split=False):
                # 2-bank energy tile; one Exp ACT covers both k-tiles
                # (the log-mask bias is per-partition and pairwise-equal
                # for every mask setup_inputs produces - host-verified)
                pts = PTS[b]
                pe = ps.tile([128, 2, BL], f32, tag="ep", bufs=2,
                             name=f"ep{b}_{kp}")
                for j in range(2):
                    kt = 2 * kp + j
                    for ct in range(2):
                        mm(pe[:, j, :],
                           k_sb[ct][:, b * BL + kt * 128:
                                    b * BL + (kt + 1) * 128],
                           q_sb[ct][:, HALF + b * BL:HALF + (b + 1) * BL],
                           ct == 0, ct == 1)
                bias = ab[:, b * NKT + 2 * kp:b * NKT + 2 * kp + 1]
                if split:
                    # two half-width exps shorten the last block's
                    # critical path into the softmax tail
                    for h in range(2):
                        nc.scalar.activation(
                            pts[:, 2 * kp:2 * kp + 2, h * HV:(h + 1) * HV],
                            pe[:, :, h * HV:(h + 1) * HV], AF.Exp,
                            bias=bias, scale=1.0 / 16.0)
                else:
                    nc.scalar.activation(
                        pts[:, 2 * kp:2 * kp + 2, :], pe[:], AF.Exp,
                        bias=bias, scale=1.0 / 16.0)

            def qk_single(b, kt):
                pts = PTS[b]
                pe = ps.tile([128, BL], f32, tag="e", bufs=4,
                             name=f"e{b}_{kt}")
                for ct in range(2):
                    mm(pe[:],
                       k_sb[ct][:, b * BL + kt * 128:
                                b * BL + (kt + 1) * 128],
                       q_sb[ct][:, HALF + b * BL:HALF + (b + 1) * BL],
                       ct == 0, ct == 1)
                nc.scalar.activation(
                    pts[:, kt, :], pe[:], AF.Exp,
                    bias=ab[:, b * NKT + kt:b * NKT + kt + 1],
                    scale=1.0 / 16.0)

            def emit_attn_head(b):
                # first two qk pairs of block b, emitted during block b-1's
                # av phase: their exps run on the ACT engine while the PE
                # finishes b-1, so b's first av never waits on an exp.
                PTS[b] = sbs.tile([128, NKT, BL], rdt, tag="pt", bufs=2,
                                  name=f"pt{b}")
                if ab_paired:
                    for kp in ((1, 2) if b == 0 else (0, 1)):
                        qk_pair(b, kp)
                else:
                    kt0 = 2 if b == 0 else 0
                    for kt in range(kt0, kt0 + 4):
                        qk_single(b, kt)

            def emit_attn_body(b):
                first = b == 0
                last = b == NBLK - 1 and ab_paired
                kt0 = 2 if first else 0  # padded k-tiles of block 0 skipped
                pts = PTS[b]
                o0 = ps.tile([128, BL], f32, tag="o0", bufs=1, name=f"o0_{b}")
                o1 = ps.tile([128, BL], f32, tag="o1", bufs=1, name=f"o1_{b}")

                def av(kt):
                    vtt = vts[(b * 4 + kt) // 2]
                    j = (b * 4 + kt) % 2
                    pt = pts[:, kt, :]
                    mm(o0[:], vtt[:, j, 0:128], pt, kt == kt0, kt == NKT - 1)
                    mm(o1[:], vtt[:, j, 128:256], pt, kt == kt0, kt == NKT - 1)

                if ab_paired:
                    if first:
                        av(2)
                        av(3)
                        qk_pair(b, 3)
                        for kt in range(4, NKT):
                            av(kt)
                    else:
                        av(0)
                        qk_pair(b, 2)
                        av(1)
                        av(2)
                        qk_pair(b, 3, split=last)
                        for kt in range(3, NKT):
                            av(kt)
                else:
                    for kt in range(kt0, NKT):
                        av(kt)
                        if kt + 4 < NKT:
                            qk_single(b, kt + 4)

                # pairwise bf16 tree for the row sums, split in halves:
                # the a-side (kt 0-3) only needs the first two exp pairs, so
                # it runs early (during av) and the post-exp critical path is
                # just the b-side + the final combine.
                a2 = sbs.tile([128, BL], rdt, tag="t2", bufs=2,
                              name=f"a2_{b}")
                if last:
                    # last block: fold three pairs (kt 0-5) into the a-side
                    # so only the kt 6/7 pair remains on the post-exp path,
                    # and run that path in halves; the sum lands in plane 0
                    # of an ep-ring-shaped PSUM tile (the pp ring stays
                    # exclusive to outproj(b-1) at the end).
                    a1 = sbs.tile([128, 3, BL], rdt, tag="t4", bufs=2,
                                  name=f"a1_{b}")
                    nc.vector.tensor_add(a1[:], pts[:, 0:6:2, :],
                                         pts[:, 1:6:2, :])
                    a2a = sbs.tile([128, BL], rdt, tag="t2a", bufs=2,
                                   name=f"a2a_{b}")
                    nc.vector.tensor_add(a2a[:], a1[:, 0, :], a1[:, 1, :])
                    nc.vector.tensor_add(a2[:], a2a[:], a1[:, 2, :])
                    sp = ps.tile([128, 2, BL], f32, tag="ep", bufs=2,
                                 name=f"s{b}")
                    for h in range(2):
                        hs = slice(h * HV, (h + 1) * HV)
                        b2 = sbs.tile([128, HV], rdt, tag="t2b", bufs=2,
                                      name=f"b2_{b}_{h}")
                        nc.vector.tensor_add(b2[:], pts[:, 6, hs],
                                             pts[:, 7, hs])
                        sar = sbs.tile([128, HV], rdt, tag="sar", bufs=2,
                                       name=f"sar{b}_{h}")
                        nc.vector.tensor_add(sar[:], a2[:, hs], b2[:])
                        mm(sp[:, 0, hs], ones_m[:], sar[:], True, True)
                elif first:
                    nc.vector.tensor_add(a2[:], pts[:, 2, :], pts[:, 3, :])
                else:
                    a1 = sbs.tile([128, 2, BL], rdt, tag="t4", bufs=2,
                                  name=f"a1_{b}")
                    nc.vector.tensor_add(a1[:], pts[:, 0:4:2, :],
                                         pts[:, 1:4:2, :])
                    nc.vector.tensor_add(a2[:], a1[:, 0, :], a1[:, 1, :])
                if not last:
                    b1 = sbs.tile([128, 2, BL], rdt, tag="t4b", bufs=2,
                                  name=f"b1_{b}")
                    nc.vector.tensor_add(b1[:], pts[:, 4:NKT:2, :],
                                         pts[:, 5:NKT:2, :])
                    b2 = sbs.tile([128, BL], rdt, tag="t2b", bufs=2,
                                  name=f"b2_{b}")
                    nc.vector.tensor_add(b2[:], b1[:, 0, :], b1[:, 1, :])
                    sar = sbs.tile([128, BL], rdt, tag="sar", bufs=2,
                                   name=f"sar{b}")
                    nc.vector.tensor_add(sar[:], a2[:], b2[:])
                    SAR[b] = sar
                else:
                    SPS[b] = sp
                OPS[b] = (o0, o1)

            def emit_sum(b):
                # [128,128] ones lhsT: reduces partitions AND replicates
                # the row-sum to all 128 partitions. Emitted after block
                # b+1's qk head so the PE reaches it well after the DVE
                # tree has produced sar. (A GpSimd partition_all_reduce was
                # tried instead and is ~30us slower overall - its ucode cost
                # on [128,512] dwarfs the 216ns matmul.)
                sp = ps.tile([128, BL], f32, tag="pp", bufs=2, name=f"s{b}")
                mm(sp[:], ones_m[:], SAR[b][:], True, True)
                SPS[b] = sp

            SAR = {}

            def emit_finA(b, h=None):
                hs = slice(0, BL) if h is None else slice(h * HV, (h + 1) * HV)
                if h in (None, 0):
                    rb = sbs.tile([128, BL], f32, tag="rbs", bufs=2,
                                  name=f"rb{b}")
                    RBS[b] = rb
                tail = b == NBLK - 1 and ab_paired
                src = SPS[b][:, 0, hs] if tail else SPS[b][:, hs]
                nc.vector.reciprocal_approx_fast(RBS[b][:, hs], src)

            def emit_normrelu(b, h=None):
                hs = slice(0, BL) if h is None else slice(h * HV, (h + 1) * HV)
                if h in (None, 0):
                    ORL[b] = [
                        sbs.tile([128, BL], rdt, tag=f"rl{m}", bufs=2,
                                 name=f"rl{b}_{m}")
                        for m in range(2)]
                for m in range(2):
                    rl = ORL[b][m]
                    if bv_zero:
                        # relu(o/s + 0) == relu(o) * (1/s)   (s > 0)
                        nc.vector.scalar_tensor_tensor(
                            rl[:, hs], OPS[b][m][:, hs], 0.0, RBS[b][:, hs],
                            ALU.max, ALU.mult)
                    else:
                        on = sbs.tile([128, BL], f32, tag=f"on{m}", bufs=2,
                                      name=f"on{b}_{m}_{h}")
                        nc.vector.tensor_mul(on[:, hs], OPS[b][m][:, hs],
                                             RBS[b][:, hs])
                        nc.vector.tensor_scalar(
                            rl[:, hs], on[:, hs], bv[:, m, :], 0.0,
                            ALU.add, ALU.max)

            def emit_outproj(b):
                ob = OB[b] = sbs.tile([128, 4, BL], rdt, tag="ob", bufs=2,
                                      name=f"ob{b}")
                for v in range(4):
                    po = ps.tile([128, BL], f32, tag="pp", bufs=2,
                                 name=f"po{b}_{v}")
                    for m in range(2):
                        mm(po[:], wo[:, m, v * 128:(v + 1) * 128],
                           ORL[b][m][:], m == 0, m == 1)
                    # bo is added on the host; spread the evacuation copies
                    # across engines (gpsimd cannot read PSUM), and DMA each
                    # v-slice as it lands so the tail transfer is small
                    # for the second-to-last block ALL copies go to the
                    # scalar engine: the DVE must stay clear for the last
                    # block's softmax tail chain.
                    if v in (1, 3) or (b == NBLK - 2 and ab_paired):
                        nc.scalar.copy(ob[:, v, :], po[:])
                    else:
                        nc.vector.tensor_copy(ob[:, v, :], po[:])
                    dma(out=out_r[:, v:v + 1, b * BL:(b + 1) * BL],
                        in_=ob[:, v:v + 1, :])

            def emit_tail_outproj_half(b, h):
                # last block: each (v-pair, half) quarter gets its OWN
                # ep-ring-shaped PSUM tile (shared tiles would create
                # tile-level false deps between the h0 copies and the h1
                # matmuls). h0 copies go to the scalar engine (free after
                # the exps), h1 copies to the DVE (free after its softmax
                # chain).
                hs = slice(h * HV, (h + 1) * HV)
                ob = OB[b]
                for vp in range(2):
                    po = ps.tile([128, 2, BL], f32, tag="ep", bufs=2,
                                 name=f"tpo{b}_{h}_{vp}")
                    for j in range(2):
                        v = vp * 2 + j
                        for m in range(2):
                            mm(po[:, j, 0:HV],
                               wo[:, m, v * 128:(v + 1) * 128],
                               ORL[b][m][:, hs], m == 0, m == 1)
                    if h == 0:
                        nc.scalar.copy(ob[:, 2 * vp:2 * vp + 2, hs],
                                       po[:, :, 0:HV])
                    else:
                        nc.vector.tensor_copy(ob[:, 2 * vp:2 * vp + 2, hs],
                                              po[:, :, 0:HV])
                    dma(out=out_r[:, 2 * vp:2 * vp + 2, b * BL + hs.start:
                                  b * BL + hs.stop],
                        in_=ob[:, 2 * vp:2 * vp + 2, hs])

            OB = {}

            # block 0's qk head is emitted mid-projections (its k/q inputs
            # are ready after chunk 2) so its exps are long done when the
            # attention phase starts.
            emit_attn_head(0)
            for c in range(3, NCH):
                emit_chunk(c)

            for step in range(NBLK + 1):
                if step == NBLK:
                    # pipeline drain: keep the PE pstate hot with dummies so
                    # the last outproj runs at full clock instead of the
                    # cold 2-3x-slower rate. The first dummy reads the
                    # previous block's evacuated output so the scheduler
                    # cannot hoist the drain ahead of that outproj.
                    dps = ps.tile([128, BL], f32, tag="pp", bufs=2,
                                  name="dummy")
                    ndrain = (8, 4) if ab_paired else (16, 10)
                    anchor = OB[NBLK - 2][:, 0, 0:256]
                    for i in range(ndrain[0]):
                        nc.tensor.matmul(dps[:, 0:256], wrm[:, 0:128],
                                         anchor if i == 0 else wrm[:, 0:256],
                                         start=True, stop=True)
                    for i in range(ndrain[1]):
                        nc.tensor.matmul(dps[:, 0:64], wrm[:, 0:128],
                                         wrm[:, 0:64], start=True, stop=True)
                if 1 <= step <= NBLK:
                    if step == NBLK and ab_paired:
                        # last block: halves-pipelined tail. Both recips are
                        # queued first (so the sum tile's ep-ring slot frees
                        # early), then the stts; the h0 outproj matmuls and
                        # scalar copies overlap the DVE's h1 chain.
                        b = step - 1
                        OB[b] = sbs.tile([128, 4, BL], rdt, tag="ob",
                                         bufs=2, name=f"ob{b}")
                        emit_finA(b, 0)
                        emit_finA(b, 1)
                        emit_normrelu(b, 0)
                        emit_normrelu(b, 1)
                        emit_tail_outproj_half(b, 0)
                        emit_tail_outproj_half(b, 1)
                    else:
                        emit_normrelu(step - 1)
                if step < NBLK:
                    emit_attn_body(step)
                    if step + 1 < NBLK:
                        emit_attn_head(step + 1)
                    if not (step == NBLK - 1 and ab_paired):
                        emit_sum(step)
                if step < NBLK:
                    if step == NBLK - 1 and ab_paired:
                        pass  # finA folded into the tail halves above
                    else:
                        emit_finA(step)
                if 1 <= step <= NBLK:
                    if step == NBLK and ab_paired:
                        pass  # outproj folded into the tail halves above
                    else:
                        emit_outproj(step - 1)

    nc.compile()
    return nc


def get_nc(bv_zero=True, ab_paired=True):
    key = ("bf16", bv_zero, ab_paired)
    if key not in _NC_CACHE:
        _NC_CACHE[key] = _build_nc(bv_zero, ab_paired)
    return _NC_CACHE[key]


def make_core_inputs(inputs):
    """Split full inputs into 8 per-core input maps.

    Cores 2h+1 (second sequence half) get their slice REVERSED along L so
    the zero-padded sequence edge is the left halo on every core (the
    sliding-window attention is reflection-symmetric). assemble_output
    un-reverses.
    """
    import ml_dtypes
    bf16 = ml_dtypes.bfloat16

    x1 = np.asarray(inputs["x1"], dtype=np.float32)
    mask = np.asarray(inputs["mask"], dtype=np.float32)
    def pack_w(w, groups):
        # [Cin, Cout] -> transposed, partition-packed [128, groups, Cout]
        wt = np.asarray(w, np.float32).T.astype(bf16)
        return np.ascontiguousarray(
            wt.reshape(groups, 128, wt.shape[1]).transpose(1, 0, 2))

    wq_t = pack_w(inputs["Wq"], 4)
    wk_t = pack_w(inputs["Wk"], 4)
    wv_t = pack_w(inputs["Wv"], 4)
    wo_t = pack_w(inputs["Wo"], 2)
    bq = np.asarray(inputs["bq"], np.float32).reshape(C, 1)
    bk = np.asarray(inputs["bk"], np.float32).reshape(C, 1)
    bv = np.asarray(inputs["bv"], np.float32).reshape(C, 1)

    in_maps = []
    for core in range(NCORES):
        b, h = divmod(core, 2)
        # core-extended input cols j in [0, LEXT); j==HALF is the first
        # owned frame. h==0: frame(j) = j - HALF; h==1 (reversed):
        # frame(j) = (L - 1 + HALF) - j. Out-of-range -> zero pad (always
        # the LEFT halo j < HALF thanks to the mirror trick).
        xe = np.zeros((CIN, LEXT), bf16)
        me = np.zeros((LEXT,), np.float32)
        if h == 0:
            xe[:, HALF:] = x1[b, :, 0:LCH + HALF].astype(bf16)
            me[HALF:] = mask[b, 0, 0:LCH + HALF]
        else:
            xe[:, HALF:] = x1[b, :, LCH - HALF:L][:, ::-1].astype(bf16)
            me[HALF:] = mask[b, 0, LCH - HALF:L][::-1]
        lbc = np.log(me + np.float32(1e-6))
        # pack chunk-contiguous: [NCH, 128, 4, BL]
        xe = np.ascontiguousarray(
            xe.reshape(4, 128, NCH, BL).transpose(2, 1, 0, 3))
        ab = np.empty((128, NBLK * NKT), np.float32)
        for blk in range(NBLK):
            w = lbc[blk * BL:blk * BL + WS]
            ab[:, blk * NKT:(blk + 1) * NKT] = w.reshape(NKT, 128).T
        in_maps.append({
            "x": xe, "wq_t": wq_t, "wk_t": wk_t, "wv_t": wv_t, "wo_t": wo_t,
            "bq": bq, "bk": bk, "bv": bv, "abias": ab,
        })
    return in_maps


def assemble_output(results, bo):
    out = np.empty((B, VD, L), np.float32)
    bo_col = np.asarray(bo, np.float32).reshape(VD, 1)
    for core in range(NCORES):
        b, h = divmod(core, 2)
        o = results[core]["out"].astype(np.float32)
        if h == 1:
            o = o[:, ::-1]
        out[b, :, h * LCH:(h + 1) * LCH] = o + bo_col
    return out


LAST_RESULT = None


def kernel(**inputs):
    global LAST_RESULT
    from concourse.bass_utils import run_bass_kernel_spmd

    bv_zero = bool(np.all(np.asarray(inputs["bv"]) == 0.0))
    in_maps = make_core_inputs(inputs)
    ab_paired = all(
        np.array_equal(m["abias"][:, 0::2], m["abias"][:, 1::2])
        for m in in_maps)
    nc = get_nc(bv_zero, ab_paired)
    res = run_bass_kernel_spmd(nc, in_maps, list(range(NCORES)))
    LAST_RESULT = res
    return assemble_output(res.results, inputs["bo"])


# revision 33
# speedup vs baseline: 1.2491x; 1.0243x over previous
"""Trainium2 Bass kernel for nn_AttLayer (sliding-block attention encoder layer).

Sharding: 8 cores = 4 batches x 2 sequence halves (4096 frames each).
Each core gets its x1 slice with a 256-frame halo on both sides, computes
q/k/v projections, 8 blocks of windowed attention (block 512, window 1024),
relu + output projection locally. No collectives.

Mirror trick: sliding-window attention is reflection-symmetric (window =
block +-256), so the second-half cores get their sequence REVERSED on the
host. Every core then sees the zero-padded sequence edge as its LEFT halo
and real neighbor data as its RIGHT halo, letting one SPMD program skip all
work on the zero halo (k/v projection of cols [0,256) and the two padded
k-tiles of block 0's window) whose contribution to the reference output is
exactly zero (padded mask -> att*mb == 0, padded v == bias).

Device layout choices:
  - all matmul operands in BF16 (x, weights host-cast; q/k/pt/v/rl via
    PSUM-evacuation output dtype). PSUM accumulation stays fp32.
  - q, k stored [c=256(2 ptiles), Lext=4608] in SBUF.
  - v stored TRANSPOSED [Lext(36 ptiles), c3=256]  (computed directly as
    x^T @ Wv^T so no on-chip transpose is ever needed).
  - energy computed transposed: eT[k, q] = sum_c k[c,k] q[c,q]  -> the softmax
    log-mask bias lands on the partition dim, a perfect fit for the ACT
    engine's per-partition bias operand:  P = Exp(eT/16 + bias).
  - no max-subtraction in softmax (energies are O(10), exp is safe in fp32).
  - row sums over the exp tiles via a pairwise bf16 tree on DVE, then a
    [128,128] ones matmul that reduces partitions AND replicates the sum to
    all 128 partitions; reciprocal runs directly on that PSUM.
  - relu & normalization fused in one DVE op via the identity
    relu(o/s) = relu(o)*(1/s):   rl = (o max 0) * rb   (scalar_tensor_tensor).
  - an early dummy Exp on the ACT engine triggers the one-time
    ACT_TABLE_LOAD (~1.3us) during the DMA head instead of right when the
    first PSUM evacuation is needed.
  - PE warmup dummies (issued as early as possible: wrm is memset on the
    otherwise idle GpSimd engine which boots first) ramp the HAM clock gate
    during the DMA head; keep-warm dummies in the drain window hold the
    clock for the last block's output projection.
  - the last block's softmax tail / normrelu / output projection run in
    256-col halves so the final DVE chain overlaps PE work (shorter tail).
"""

import numpy as np

# problem constants (self-contained; must match the harness reference)
B, CIN, L = 4, 512, 8192
C, VD = 256, 512
BL, HALF = 512, 256
NCORES = 8
LCH = L // 2            # 4096 frames per core
LEXT = LCH + 2 * HALF   # 4608 with halo
NBLK = LCH // BL        # 8 local blocks
WS = BL + 2 * HALF      # 1024 window
NKT = WS // 128         # 8 k-tiles per window
NCH = LEXT // BL        # 9 x chunks
NVT = LEXT // 128       # 36 v^T partition tiles

_NC_CACHE = {}


def _build_nc(bv_zero, ab_paired=True):
    import concourse.bacc as bacc
    import concourse.mybir as mybir
    import concourse.tile as tile
    from contextlib import ExitStack

    f32 = mybir.dt.float32
    bf16 = mybir.dt.bfloat16
    rdt = bf16
    AF = mybir.ActivationFunctionType
    ALU = mybir.AluOpType

    nc = bacc.Bacc("TRN2", target_bir_lowering=False, debug=False,
                   num_devices=NCORES)

    # x and weights are host-packed so every DMA is a single contiguous
    # per-partition read (4KB packets instead of 4x1KB strided segments -
    # much faster on the cold DMA path at kernel start)
    x_d = nc.dram_tensor("x", [NCH, 128, 4, BL], rdt,
                         kind="ExternalInput").ap()
    wq_d = nc.dram_tensor("wq_t", [128, 4, C], rdt, kind="ExternalInput").ap()
    wk_d = nc.dram_tensor("wk_t", [128, 4, C], rdt, kind="ExternalInput").ap()
    wv_d = nc.dram_tensor("wv_t", [128, 4, C], rdt, kind="ExternalInput").ap()
    wo_d = nc.dram_tensor("wo_t", [128, 2, VD], rdt,
                          kind="ExternalInput").ap()
    bq_d = nc.dram_tensor("bq", [C, 1], f32, kind="ExternalInput").ap()
    bk_d = nc.dram_tensor("bk", [C, 1], f32, kind="ExternalInput").ap()
    bv_d = nc.dram_tensor("bv", [C, 1], f32, kind="ExternalInput").ap()
    ab_d = nc.dram_tensor("abias", [128, NBLK * NKT], f32,
                          kind="ExternalInput").ap()
    out_d = nc.dram_tensor("out", [VD, LCH], rdt, kind="ExternalOutput").ap()

    wq_r, wk_r, wv_r, wo_r = wq_d, wk_d, wv_d, wo_d
    bq_r = bq_d.rearrange("(m p) o -> p m o", p=128)    # [128, 2, 1]
    bk_r = bk_d.rearrange("(m p) o -> p m o", p=128)
    bv_r = bv_d.rearrange("(m p) o -> p m o", p=128)
    out_r = out_d.rearrange("(v p) l -> p v l", p=128)  # [128, 4, 4096]

    with tile.TileContext(nc) as tc:
        with ExitStack() as ctx:
            ctx.enter_context(nc.allow_low_precision(
                reason="bf16 matmul pipeline; fp32 PSUM accumulation"))
            sbc = ctx.enter_context(tc.tile_pool(name="sbc", bufs=1))  # constants
            sbp = ctx.enter_context(tc.tile_pool(name="sbp", bufs=1))  # persistent
            sbs = ctx.enter_context(tc.tile_pool(name="sbs", bufs=1))  # streaming
            ps = ctx.enter_context(tc.tile_pool(name="ps", bufs=1, space="PSUM"))

            dma = nc.sync.dma_start

            # warmup operand, memset on GpSimd (idle engine, earliest boot)
            wrm = sbc.tile([128, BL], bf16, tag="wrm", name="wrm")
            nc.gpsimd.memset(wrm[:], 0.0)
            # trigger the one-time ACT table load (~1.3us) during the boot
            # window. The bias must be a memset SBUF tile, NOT a float
            # constant: a float would allocate a const-AP whose init DMA
            # queues ahead of the x/weight transfers and delays them.
            zb = sbc.tile([128, 1], f32, tag="zb", name="zb")
            nc.gpsimd.memset(zb[:], 0.0)
            dact = sbc.tile([128, 1], f32, tag="dact", name="dact")
            nc.scalar.activation(dact[:], zb[:], AF.Exp, bias=zb[:],
                                 scale=1.0)

            wq = sbc.tile([128, 4, C], rdt, tag="wq", name="wq")
            wk = sbc.tile([128, 4, C], rdt, tag="wk", name="wk")
            wv = sbc.tile([128, 4, C], rdt, tag="wv", name="wv")
            xt0 = sbs.tile([128, 4, BL], rdt, tag="x", bufs=3, name="xt0")
            xt1 = sbs.tile([128, 4, BL], rdt, tag="x", bufs=3, name="xt1")
            bq = sbc.tile([128, 2, 1], f32, tag="bq", name="bq")
            bk = sbc.tile([128, 2, 1], f32, tag="bk", name="bk")
            # (issuing these from the GpSimd queue was tried - its DMA path
            # has ~6us more latency than the Sync engine's, a clear loss)
            dma(out=xt0[:], in_=x_d[0])
            dma(out=wq[:], in_=wq_r)
            dma(out=bq[:], in_=bq_r)
            dma(out=wk[:], in_=wk_r)
            dma(out=wv[:], in_=wv_r)
            dma(out=bk[:], in_=bk_r)
            dma(out=xt1[:], in_=x_d[1])
            xt2 = sbs.tile([128, 4, BL], rdt, tag="x", bufs=3, name="xt2")
            dma(out=xt2[:], in_=x_d[2])

            # PE warmup: dependency-free bf16 matmuls during the DMA head
            # so the HAM clock-gate reaches 8/8 before real work arrives.
            wps = ps.tile([128, BL], f32, tag="pp", bufs=2, name="wps")
            for i in range(10):
                nc.tensor.matmul(wps[:], wrm[:, 0:128], wrm[:], start=True,
                                 stop=True)

            def emit_late_consts():
                wo = sbc.tile([128, 2, VD], rdt, tag="wo", name="wo")
                dma(out=wo[:], in_=wo_r)
                bv = sbc.tile([128, 2, 1], f32, tag="bv", name="bv")
                dma(out=bv[:], in_=bv_r)
                ab = sbc.tile([128, NBLK * NKT], f32, tag="ab", name="ab")
                dma(out=ab[:], in_=ab_d)
                ones_f = sbc.tile([128, 128], f32, tag="ones_f", name="ones_f")
                nc.vector.memset(ones_f[:], 1.0)
                ones_m = sbc.tile([128, 128], rdt, tag="ones_m", name="ones_m")
                nc.vector.tensor_copy(ones_m[:], ones_f[:])
                return wo, bv, ab, ones_m

            q_sb = [sbp.tile([128, LEXT], rdt, tag=f"qsb{i}", name=f"qsb{i}")
                    for i in range(2)]
            k_sb = [sbp.tile([128, LEXT], rdt, tag=f"ksb{i}", name=f"ksb{i}")
                    for i in range(2)]
            # v^T tiles, paired: [128, 2, C] so the PSUM evacuation is one op
            vts = [sbp.tile([128, 2, C], rdt, tag=f"vt{i}", name=f"vt{i}")
                   for i in range(NVT // 2)]

            def mm(out_ap, lhsT, rhs, start, stop):
                nc.tensor.matmul(out_ap, lhsT, rhs, start=start, stop=stop)

            # paired-exp builds use 2-bank "ep" slots (2 bufs); the fallback
            # uses 1-bank "e" slots (4 bufs). One tag per build keeps the
            # PSUM budget at 8 banks: pp(2) + energy(4) + o0 + o1.
            ptag = "ep" if ab_paired else "e"
            pbufs = 2 if ab_paired else 4

            # ---------------- projections (streamed over 9 x-chunks) --------
            # chunk 0 skips cols [0, HALF) for k and v: that is the
            # zero-padded sequence edge (mirror trick) and no surviving
            # attention window reads it.
            def emit_chunk(c):
                if c == 0:
                    xt = xt0
                elif c == 1:
                    xt = xt1
                elif c == 2:
                    xt = xt2
                else:
                    xt = sbs.tile([128, 4, BL], rdt, tag="x", bufs=3,
                                  name=f"xt{c}")
                    dma(out=xt[:], in_=x_d[c])
                # q is only needed on extended cols [HALF, LEXT-HALF)
                qlo = max(c * BL, HALF) - c * BL
                qhi = min((c + 1) * BL, LEXT - HALF) - c * BL
                klo = HALF if c == 0 else 0
                for o in range(2):
                    pq = ps.tile([128, BL], f32, tag="pp", bufs=2,
                                 name=f"pq{c}_{o}")
                    for r in range(4):
                        mm(pq[:, 0:qhi - qlo],
                           wq[:, r, o * 128:(o + 1) * 128],
                           xt[:, r, qlo:qhi], r == 0, r == 3)
                    # split the two q evacuations across scalar/vector so
                    # the pp-ring drains in parallel
                    if o == 0:
                        nc.scalar.activation(
                            q_sb[o][:, c * BL + qlo:c * BL + qhi],
                            pq[:, 0:qhi - qlo], AF.Identity,
                            bias=bq[:, o, :], scale=1.0)
                    else:
                        nc.vector.tensor_scalar_add(
                            q_sb[o][:, c * BL + qlo:c * BL + qhi],
                            pq[:, 0:qhi - qlo], bq[:, o, :])
                # k PSUM groups live on the ep ring (free during the
                # projection phase): this decouples them from the q groups'
                # pp ring so neither waits on the other's evacuation.
                for o in range(2):
                    pk = ps.tile([128, BL], f32, tag=ptag, bufs=pbufs,
                                 name=f"pk{c}_{o}")
                    for r in range(4):
                        mm(pk[:, 0:BL - klo],
                           wk[:, r, o * 128:(o + 1) * 128],
                           xt[:, r, klo:BL], r == 0, r == 3)
                    nc.vector.tensor_scalar_add(
                        k_sb[o][:, c * BL + klo:(c + 1) * BL],
                        pk[:, 0:BL - klo], bk[:, o, :])
                for lp in range(2):
                    if c == 0 and lp == 0:
                        continue  # zero-halo v tiles: never read
                    pv = ps.tile([128, 2, C], f32, tag=ptag, bufs=pbufs,
                                 name=f"pv{c}_{lp}")
                    for j in range(2):
                        lt = lp * 2 + j
                        for r in range(4):
                            mm(pv[:, j, :], xt[:, r, lt * 128:(lt + 1) * 128],
                               wv[:, r, :], r == 0, r == 3)
                    nc.vector.tensor_copy(vts[c * 2 + lp][:], pv[:])

            emit_chunk(0)
            wo, bv, ab, ones_m = emit_late_consts()
            emit_chunk(1)
            emit_chunk(2)

            # ---------------- attention (software-pipelined blocks) ---------
            OPS, SPS, RBS, ORL, PTS = {}, {}, {}, {}, {}
            HV = BL // 2  # half width for the last block's tail split

            def qk_pair(b, kp, split=False):
                # 2-bank energy tile; one Exp ACT covers both k-tiles
                # (the log-mask bias is per-partition and pairwise-equal
                # for every mask setup_inputs produces - host-verified)
                pts = PTS[b]
                pe = ps.tile([128, 2, BL], f32, tag="ep", bufs=2,
                             name=f"ep{b}_{kp}")
                for j in range(2):
                    kt = 2 * kp + j
                    for ct in range(2):
                        mm(pe[:, j, :],
                           k_sb[ct][:, b * BL + kt * 128:
                                    b * BL + (kt + 1) * 128],
                           q_sb[ct][:, HALF + b * BL:HALF + (b + 1) * BL],
                           ct == 0, ct == 1)
                bias = ab[:, b * NKT + 2 * kp:b * NKT + 2 * kp + 1]
                if split:
                    # two half-width exps shorten the last block's
                    # critical path into the softmax tail
                    for h in range(2):
                        nc.scalar.activation(
                            pts[:, 2 * kp:2 * kp + 2, h * HV:(h + 1) * HV],
                            pe[:, :, h * HV:(h + 1) * HV], AF.Exp,
                            bias=bias, scale=1.0 / 16.0)
                else:
                    nc.scalar.activation(
                        pts[:, 2 * kp:2 * kp + 2, :], pe[:], AF.Exp,
                        bias=bias, scale=1.0 / 16.0)

            def qk_single(b, kt):
                pts = PTS[b]
                pe = ps.tile([128, BL], f32, tag="e", bufs=4,
                             name=f"e{b}_{kt}")
                for ct in range(2):
                    mm(pe[:],
                       k_sb[ct][:, b * BL + kt * 128:
                                b * BL + (kt + 1) * 128],
                       q_sb[ct][:, HALF + b * BL:HALF + (b + 1) * BL],
                       ct == 0, ct == 1)
                nc.scalar.activation(
                    pts[:, kt, :], pe[:], AF.Exp,
                    bias=ab[:, b * NKT + kt:b * NKT + kt + 1],
                    scale=1.0 / 16.0)

            def emit_attn_head(b):
                # first two qk pairs of block b, emitted during block b-1's
                # av phase: their exps run on the ACT engine while the PE
                # finishes b-1, so b's first av never waits on an exp. The
                # LAST block gets all four pairs in its head - its whole
                # softmax chain (tree/recip/stt on DVE) then overlaps the
                # av phase, collapsing the kernel tail.
                PTS[b] = sbs.tile([128, NKT, BL], rdt, tag="pt", bufs=2,
                                  name=f"pt{b}")
                if ab_paired:
                    if b == NBLK - 1:
                        for kp in range(3):
                            qk_pair(b, kp)
                        qk_pair(b, 3, split=True)
                    else:
                        for kp in ((1, 2) if b == 0 else (0, 1)):
                            qk_pair(b, kp)
                else:
                    kt0 = 2 if b == 0 else 0
                    for kt in range(kt0, kt0 + 4):
                        qk_single(b, kt)

            def emit_attn_body(b):
                first = b == 0
                last = b == NBLK - 1 and ab_paired
                kt0 = 2 if first else 0  # padded k-tiles of block 0 skipped
                pts = PTS[b]
                o0 = ps.tile([128, BL], f32, tag="o0", bufs=1, name=f"o0_{b}")
                o1 = ps.tile([128, BL], f32, tag="o1", bufs=1, name=f"o1_{b}")

                def av(kt):
                    vtt = vts[(b * 4 + kt) // 2]
                    j = (b * 4 + kt) % 2
                    pt = pts[:, kt, :]
                    mm(o0[:], vtt[:, j, 0:128], pt, kt == kt0, kt == NKT - 1)
                    mm(o1[:], vtt[:, j, 128:256], pt, kt == kt0, kt == NKT - 1)

                if ab_paired:
                    if first:
                        av(2)
                        av(3)
                        qk_pair(b, 3)
                        for kt in range(4, NKT):
                            av(kt)
                    elif last:
                        # all four pairs were emitted in this block's head
                        for kt in range(NKT):
                            av(kt)
                    else:
                        av(0)
                        qk_pair(b, 2)
                        av(1)
                        av(2)
                        qk_pair(b, 3)
                        for kt in range(3, NKT):
                            av(kt)
                else:
                    for kt in range(kt0, NKT):
                        av(kt)
                        if kt + 4 < NKT:
                            qk_single(b, kt + 4)

                # pairwise bf16 tree for the row sums, split in halves:
                # the a-side (kt 0-3) only needs the first two exp pairs, so
                # it runs early (during av) and the post-exp critical path is
                # just the b-side + the final combine.
                a2 = sbs.tile([128, BL], rdt, tag="t2", bufs=2,
                              name=f"a2_{b}")
                if last:
                    # last block: fold three pairs (kt 0-5) into the a-side
                    # so only the kt 6/7 pair remains on the post-exp path,
                    # and run that path in halves; the sum lands in plane 0
                    # of an ep-ring-shaped PSUM tile (the pp ring stays
                    # exclusive to outproj(b-1) at the end).
                    a1 = sbs.tile([128, 3, BL], rdt, tag="t4", bufs=2,
                                  name=f"a1_{b}")
                    nc.vector.tensor_add(a1[:], pts[:, 0:6:2, :],
                                         pts[:, 1:6:2, :])
                    a2a = sbs.tile([128, BL], rdt, tag="t2a", bufs=2,
                                   name=f"a2a_{b}")
                    nc.vector.tensor_add(a2a[:], a1[:, 0, :], a1[:, 1, :])
                    nc.vector.tensor_add(a2[:], a2a[:], a1[:, 2, :])
                    sp = ps.tile([128, 2, BL], f32, tag="ep", bufs=2,
                                 name=f"s{b}")
                    for h in range(2):
                        hs = slice(h * HV, (h + 1) * HV)
                        b2 = sbs.tile([128, HV], rdt, tag="t2b", bufs=2,
                                      name=f"b2_{b}_{h}")
                        nc.vector.tensor_add(b2[:], pts[:, 6, hs],
                                             pts[:, 7, hs])
                        sar = sbs.tile([128, HV], rdt, tag="sar", bufs=2,
                                       name=f"sar{b}_{h}")
                        nc.vector.tensor_add(sar[:], a2[:, hs], b2[:])
                        mm(sp[:, 0, hs], ones_m[:], sar[:], True, True)
                elif first:
                    nc.vector.tensor_add(a2[:], pts[:, 2, :], pts[:, 3, :])
                else:
                    a1 = sbs.tile([128, 2, BL], rdt, tag="t4", bufs=2,
                                  name=f"a1_{b}")
                    nc.vector.tensor_add(a1[:], pts[:, 0:4:2, :],
                                         pts[:, 1:4:2, :])
                    nc.vector.tensor_add(a2[:], a1[:, 0, :], a1[:, 1, :])
                if not last:
                    b1 = sbs.tile([128, 2, BL], rdt, tag="t4b", bufs=2,
                                  name=f"b1_{b}")
                    nc.vector.tensor_add(b1[:], pts[:, 4:NKT:2, :],
                                         pts[:, 5:NKT:2, :])
                    b2 = sbs.tile([128, BL], rdt, tag="t2b", bufs=2,
                                  name=f"b2_{b}")
                    nc.vector.tensor_add(b2[:], b1[:, 0, :], b1[:, 1, :])
                    sar = sbs.tile([128, BL], rdt, tag="sar", bufs=2,
                                   name=f"sar{b}")
                    nc.vector.tensor_add(sar[:], a2[:], b2[:])
                    SAR[b] = sar
                else:
                    SPS[b] = sp
                OPS[b] = (o0, o1)

            def emit_sum(b):
                # [128,128] ones lhsT: reduces partitions AND replicates
                # the row-sum to all 128 partitions. Emitted after block
                # b+1's qk head so the PE reaches it well after the DVE
                # tree has produced sar. (A GpSimd partition_all_reduce was
                # tried instead and is ~30us slower overall - its ucode cost
                # on [128,512] dwarfs the 216ns matmul.)
                sp = ps.tile([128, BL], f32, tag="pp", bufs=2, name=f"s{b}")
                mm(sp[:], ones_m[:], SAR[b][:], True, True)
                SPS[b] = sp

            SAR = {}

            def emit_finA(b, h=None):
                hs = slice(0, BL) if h is None else slice(h * HV, (h + 1) * HV)
                if h in (None, 0):
                    rb = sbs.tile([128, BL], f32, tag="rbs", bufs=2,
                                  name=f"rb{b}")
                    RBS[b] = rb
                tail = b == NBLK - 1 and ab_paired
                src = SPS[b][:, 0, hs] if tail else SPS[b][:, hs]
                nc.vector.reciprocal_approx_fast(RBS[b][:, hs], src)

            def emit_normrelu(b, h=None):
                hs = slice(0, BL) if h is None else slice(h * HV, (h + 1) * HV)
                if h in (None, 0):
                    ORL[b] = [
                        sbs.tile([128, BL], rdt, tag=f"rl{m}", bufs=2,
                                 name=f"rl{b}_{m}")
                        for m in range(2)]
                for m in range(2):
                    rl = ORL[b][m]
                    if bv_zero:
                        # relu(o/s + 0) == relu(o) * (1/s)   (s > 0)
                        nc.vector.scalar_tensor_tensor(
                            rl[:, hs], OPS[b][m][:, hs], 0.0, RBS[b][:, hs],
                            ALU.max, ALU.mult)
                    else:
                        on = sbs.tile([128, BL], f32, tag=f"on{m}", bufs=2,
                                      name=f"on{b}_{m}_{h}")
                        nc.vector.tensor_mul(on[:, hs], OPS[b][m][:, hs],
                                             RBS[b][:, hs])
                        nc.vector.tensor_scalar(
                            rl[:, hs], on[:, hs], bv[:, m, :], 0.0,
                            ALU.add, ALU.max)

            def emit_outproj(b):
                ob = OB[b] = sbs.tile([128, 4, BL], rdt, tag="ob", bufs=2,
                                      name=f"ob{b}")
                for v in range(4):
                    po = ps.tile([128, BL], f32, tag="pp", bufs=2,
                                 name=f"po{b}_{v}")
                    for m in range(2):
                        mm(po[:], wo[:, m, v * 128:(v + 1) * 128],
                           ORL[b][m][:], m == 0, m == 1)
                    # bo is added on the host; spread the evacuation copies
                    # across engines (gpsimd cannot read PSUM), and DMA each
                    # v-slice as it lands so the tail transfer is small
                    # for the second-to-last block ALL copies go to the
                    # scalar engine: the DVE must stay clear for the last
                    # block's softmax tail chain.
                    if v in (1, 3) or (b == NBLK - 2 and ab_paired):
                        nc.scalar.copy(ob[:, v, :], po[:])
                    else:
                        nc.vector.tensor_copy(ob[:, v, :], po[:])
                    dma(out=out_r[:, v:v + 1, b * BL:(b + 1) * BL],
                        in_=ob[:, v:v + 1, :])

            def emit_tail_outproj_half(b, h):
                # last block: each (v-pair, half) quarter gets its OWN
                # ep-ring-shaped PSUM tile (shared tiles would create
                # tile-level false deps between the h0 copies and the h1
                # matmuls). h0 copies go to the scalar engine (free after
                # the exps), h1 copies to the DVE (free after its softmax
                # chain).
                hs = slice(h * HV, (h + 1) * HV)
                ob = OB[b]
                for vp in range(2):
                    po = ps.tile([128, 2, BL], f32, tag="ep", bufs=2,
                                 name=f"tpo{b}_{h}_{vp}")
                    for j in range(2):
                        v = vp * 2 + j
                        for m in range(2):
                            mm(po[:, j, 0:HV],
                               wo[:, m, v * 128:(v + 1) * 128],
                               ORL[b][m][:, hs], m == 0, m == 1)
                    if h == 0:
                        nc.scalar.copy(ob[:, 2 * vp:2 * vp + 2, hs],
                                       po[:, :, 0:HV])
                    else:
                        nc.vector.tensor_copy(ob[:, 2 * vp:2 * vp + 2, hs],
                                              po[:, :, 0:HV])
                    dma(out=out_r[:, 2 * vp:2 * vp + 2, b * BL + hs.start:
                                  b * BL + hs.stop],
                        in_=ob[:, 2 * vp:2 * vp + 2, hs])

            OB = {}

            # block 0's qk head is emitted mid-projections (its k/q inputs
            # are ready after chunk 2) so its exps are long done when the
            # attention phase starts.
            emit_attn_head(0)
            for c in range(3, NCH):
                emit_chunk(c)

            for step in range(NBLK + 1):
                if step == NBLK:
                    # pipeline drain: keep the PE pstate hot with dummies so
                    # the last outproj runs at full clock instead of the
                    # cold 2-3x-slower rate. The first dummy reads the
                    # previous block's evacuated output so the scheduler
                    # cannot hoist the drain ahead of that outproj.
                    dps = ps.tile([128, BL], f32, tag="pp", bufs=2,
                                  name="dummy")
                    ndrain = (8, 4) if ab_paired else (16, 10)
                    anchor = OB[NBLK - 2][:, 0, 0:256]
                    for i in range(ndrain[0]):
                        nc.tensor.matmul(dps[:, 0:256], wrm[:, 0:128],
                                         anchor if i == 0 else wrm[:, 0:256],
                                         start=True, stop=True)
                    for i in range(ndrain[1]):
                        nc.tensor.matmul(dps[:, 0:64], wrm[:, 0:128],
                                         wrm[:, 0:64], start=True, stop=True)
                if 1 <= step <= NBLK:
                    if step == NBLK and ab_paired:
                        # last block: halves-pipelined tail. Both recips are
                        # queued first (so the sum tile's ep-ring slot frees
                        # early), then the stts; the h0 outproj matmuls and
                        # scalar copies overlap the DVE's h1 chain.
                        b = step - 1
                        OB[b] = sbs.tile([128, 4, BL], rdt, tag="ob",
                                         bufs=2, name=f"ob{b}")
                        emit_finA(b, 0)
                        emit_finA(b, 1)
                        emit_normrelu(b, 0)
                        emit_normrelu(b, 1)
                        emit_tail_outproj_half(b, 0)
                        emit_tail_outproj_half(b, 1)
                    else:
                        emit_normrelu(step - 1)
                if step < NBLK:
                    emit_attn_body(step)
                    if step + 1 < NBLK:
                        emit_attn_head(step + 1)
                    if not (step == NBLK - 1 and ab_paired):
                        emit_sum(step)
                if step < NBLK:
                    if step == NBLK - 1 and ab_paired:
                        pass  # finA folded into the tail halves above
                    else:
                        emit_finA(step)
                if 1 <= step <= NBLK:
                    if step == NBLK and ab_paired:
                        pass  # outproj folded into the tail halves above
                    else:
                        emit_outproj(step - 1)

    nc.compile()
    return nc


def get_nc(bv_zero=True, ab_paired=True):
    key = ("bf16", bv_zero, ab_paired)
    if key not in _NC_CACHE:
        _NC_CACHE[key] = _build_nc(bv_zero, ab_paired)
    return _NC_CACHE[key]


def make_core_inputs(inputs):
    """Split full inputs into 8 per-core input maps.

    Cores 2h+1 (second sequence half) get their slice REVERSED along L so
    the zero-padded sequence edge is the left halo on every core (the
    sliding-window attention is reflection-symmetric). assemble_output
    un-reverses.
    """
    import ml_dtypes
    bf16 = ml_dtypes.bfloat16

    x1 = np.asarray(inputs["x1"], dtype=np.float32)
    mask = np.asarray(inputs["mask"], dtype=np.float32)
    def pack_w(w, groups):
        # [Cin, Cout] -> transposed, partition-packed [128, groups, Cout]
        wt = np.asarray(w, np.float32).T.astype(bf16)
        return np.ascontiguousarray(
            wt.reshape(groups, 128, wt.shape[1]).transpose(1, 0, 2))

    wq_t = pack_w(inputs["Wq"], 4)
    wk_t = pack_w(inputs["Wk"], 4)
    wv_t = pack_w(inputs["Wv"], 4)
    wo_t = pack_w(inputs["Wo"], 2)
    bq = np.asarray(inputs["bq"], np.float32).reshape(C, 1)
    bk = np.asarray(inputs["bk"], np.float32).reshape(C, 1)
    bv = np.asarray(inputs["bv"], np.float32).reshape(C, 1)

    in_maps = []
    for core in range(NCORES):
        b, h = divmod(core, 2)
        # core-extended input cols j in [0, LEXT); j==HALF is the first
        # owned frame. h==0: frame(j) = j - HALF; h==1 (reversed):
        # frame(j) = (L - 1 + HALF) - j. Out-of-range -> zero pad (always
        # the LEFT halo j < HALF thanks to the mirror trick).
        xe = np.zeros((CIN, LEXT), bf16)
        me = np.zeros((LEXT,), np.float32)
        if h == 0:
            xe[:, HALF:] = x1[b, :, 0:LCH + HALF].astype(bf16)
            me[HALF:] = mask[b, 0, 0:LCH + HALF]
        else:
            xe[:, HALF:] = x1[b, :, LCH - HALF:L][:, ::-1].astype(bf16)
            me[HALF:] = mask[b, 0, LCH - HALF:L][::-1]
        lbc = np.log(me + np.float32(1e-6))
        # pack chunk-contiguous: [NCH, 128, 4, BL]
        xe = np.ascontiguousarray(
            xe.reshape(4, 128, NCH, BL).transpose(2, 1, 0, 3))
        ab = np.empty((128, NBLK * NKT), np.float32)
        for blk in range(NBLK):
            w = lbc[blk * BL:blk * BL + WS]
            ab[:, blk * NKT:(blk + 1) * NKT] = w.reshape(NKT, 128).T
        in_maps.append({
            "x": xe, "wq_t": wq_t, "wk_t": wk_t, "wv_t": wv_t, "wo_t": wo_t,
            "bq": bq, "bk": bk, "bv": bv, "abias": ab,
        })
    return in_maps


def assemble_output(results, bo):
    out = np.empty((B, VD, L), np.float32)
    bo_col = np.asarray(bo, np.float32).reshape(VD, 1)
    for core in range(NCORES):
        b, h = divmod(core, 2)
        o = results[core]["out"].astype(np.float32)
        if h == 1:
            o = o[:, ::-1]
        out[b, :, h * LCH:(h + 1) * LCH] = o + bo_col
    return out


LAST_RESULT = None


def kernel(**inputs):
    global LAST_RESULT
    from concourse.bass_utils import run_bass_kernel_spmd

    bv_zero = bool(np.all(np.asarray(inputs["bv"]) == 0.0))
    in_maps = make_core_inputs(inputs)
    ab_paired = all(
        np.array_equal(m["abias"][:, 0::2], m["abias"][:, 1::2])
        for m in in_maps)
    nc = get_nc(bv_zero, ab_paired)
    res = run_bass_kernel_spmd(nc, in_maps, list(range(NCORES)))
    LAST_RESULT = res
    return assemble_output(res.results, inputs["bo"])
